# revision 33
# baseline (speedup 1.0000x reference)
"""Two-layer SAGEConv(mean) + PReLU GNN encoder on 8 Trainium2 NeuronCores.

v2 design (transposed dataflow):
- Nodes degree-sorted, round-robin over 8 cores; per-core local ids are
  group-major (g = local//128, p = local%128), 98 groups of 128.
- Feature table rows hold a PAIR of nodes (p, p+64) -> 256B bf16 rows, split
  into 2 chunks by source group block (g<49 / g>=49) so int16 gather indices
  address a 25088-row chunk.
- Layer-0 table is a replicated ExternalInput (no first AllGather). Layer-1
  table is AllGathered per chunk so the second half overlaps compute.
- Per (chunk, dst-group): edges are slots; dma_gather fetches pair rows; a
  batched is_equal builds the one-hot-with-parity S [128, nw*256]; two
  matmuls per window accumulate aggT [64ch, 128dst] in PSUM (chunk-0 partial
  parked in SBUF bf16, folded back via identity matmul in the chunk-1 pass).
- Epilogue (all channel-major, no per-group transpose for compute):
  saggT = aggT * scaleT (mean); hT = wlb^T @ [saggT; ones] + I^T @ hxr
  (hxr = wr^T xT precomputed per layer, bias via ones row of saggT);
  PReLU via max(v, a*v). Layer 0 additionally emits node-major h for the
  AllGather table and builds layer-1's hxr incrementally.
- Output is produced channel-major [64, 12544] f32 and untransposed on host.
"""
import sys

sys.path.insert(0, "/opt/trn_rl_repo")

import os
import numpy as np
from contextlib import ExitStack

from concourse import bass, mybir, tile, bacc, bass_utils
from concourse.masks import make_identity

P = 128
D = 64
NCORES = 8
N_NODES = 100000
SHARD_N = 12544          # 98 groups of 128 (incl 44 pad nodes)
NGROUPS = 98
GC = 49                  # groups per chunk
NCHUNK = 2
PAIRS_CC = GC * D        # 3136 pair rows per (core, chunk)
PCHUNK = NCORES * PAIRS_CC  # 25088 pair rows per chunk tensor
SG = 7                   # dst groups per gather call
NSG = GC // SG           # 7 calls per chunk... (NGROUPS//SG per layer half)
NSG_ALL = NGROUPS // SG  # 14 supergroups of dst groups
MAXIDX = 8192
PAD_OFF = 300.0          # is_equal never matches -> zero S row
F32 = mybir.dt.float32
BF16 = mybir.dt.bfloat16


# ---------------------------------------------------------------- host prep

def _prep(edge_index):
    src = np.asarray(edge_index[0], dtype=np.int64)
    dst = np.asarray(edge_index[1], dtype=np.int64)
    deg = np.bincount(dst, minlength=N_NODES)

    order = np.argsort(deg, kind="stable")
    rank = np.empty(N_NODES, np.int64)
    rank[order] = np.arange(N_NODES)
    node_core = rank % NCORES
    node_local = rank // NCORES          # [0, 12500)
    g_of = node_local // P               # dst/src group 0..97
    p_of = node_local % P

    e_core = node_core[dst]
    e_g = g_of[dst]
    e_q = p_of[dst]
    s_g = g_of[src]
    s_p = p_of[src]
    e_chunk = s_g // GC
    e_rel = node_core[src] * PAIRS_CC + (s_g % GC) * D + (s_p % D)
    e_par = s_p // D
    e_off = (e_q + 128 * e_par).astype(np.float32)
    invdeg_f = (1.0 / np.maximum(deg, 1)).astype(np.float32)
    e_scl = invdeg_f[dst]

    key = np.lexsort((e_rel, e_g, e_chunk, e_core))
    e_core = e_core[key]
    e_chunk = e_chunk[key]
    e_g = e_g[key]
    e_rel = e_rel[key]
    e_off = e_off[key]
    e_scl = e_scl[key]

    cg = (e_core * NCHUNK + e_chunk) * NGROUPS + e_g
    counts = np.bincount(cg, minlength=NCORES * NCHUNK * NGROUPS).reshape(
        NCORES, NCHUNK, NGROUPS
    )
    lens = np.maximum(((counts.max(axis=0) + P - 1) // P) * P, P)  # [NCHUNK, NGROUPS]

    slot_off = np.zeros((NCHUNK, NGROUPS), np.int64)
    run = 0
    for c in range(NCHUNK):
        for g in range(NGROUPS):
            slot_off[c, g] = run
            run += lens[c, g]
    n_slots = int(run)
    n_win = n_slots // P

    idx_all = np.zeros((NCORES, n_slots), np.int32)      # pad -> row 0
    off_all = np.full((NCORES, n_slots), PAD_OFF, np.float32)
    scl_all = np.zeros((NCORES, n_slots), np.float32)
    for core in range(NCORES):
        m = e_core == core
        ch, gr, rel, off = e_chunk[m], e_g[m], e_rel[m], e_off[m]
        scl = e_scl[m]
        kk = ch * NGROUPS + gr
        start_of_run = np.r_[True, kk[1:] != kk[:-1]]
        run_starts = np.flatnonzero(start_of_run)
        within = np.arange(kk.size) - np.repeat(
            run_starts, np.diff(np.r_[run_starts, kk.size])
        )
        pos = slot_off[ch, gr] + within
        idx_all[core, pos] = rel
        off_all[core, pos] = off
        scl_all[core, pos] = scl

    # gather calls: consecutive dst groups of one chunk, capped at CAPW windows
    CAPW = 24
    calls = []  # (chunk, g_start, n_groups, slot_start, n_slots_call)
    for c in range(NCHUNK):
        g = 0
        while g < NGROUPS:
            g0 = g
            w = 0
            while g < NGROUPS and w + lens[c, g] // P <= CAPW:
                w += lens[c, g] // P
                g += 1
            calls.append((c, g0, g - g0, int(slot_off[c, g0]), int(w * P)))
    # taper the tail: split the final call into per-2-group calls
    last = calls.pop()
    c_, g0_, ng_, s0_, _ = last
    g = g0_
    while g < g0_ + ng_:
        ge = min(g + 2, g0_ + ng_)
        w = int(lens[c_, g:ge].sum())
        calls.append((c_, g, ge - g, int(slot_off[c_, g]), w))
        g = ge

    idx_wrap = np.empty((NCORES, 16, n_slots // 16), np.int16)
    seg = idx_all.reshape(NCORES, n_slots // 16, 16)
    idx_wrap[:] = seg.transpose(0, 2, 1)
    offw = off_all.reshape(NCORES, n_win, P).transpose(0, 2, 1)  # [., 128, n_win]

    import ml_dtypes
    scaleT = np.zeros((NCORES, SHARD_N), np.float32)
    scaleT[node_core, node_local] = invdeg_f
    scaleT = np.repeat(scaleT[:, None, :], D, axis=1).astype(ml_dtypes.bfloat16)

    meta = {
        "lens": lens,
        "slot_off": slot_off,
        "calls": calls,
        "max_call_win": CAPW,
        "n_slots": n_slots,
        "n_win": n_win,
    }
    percore = {
        "idx_wrap": idx_wrap,
        "offw": offw,
        "scaleT": scaleT,
        "node_core": node_core,
        "node_local": node_local,
        "g_of": g_of,
        "p_of": p_of,
    }
    return meta, percore


# ------------------------------------------------------------- bass program

def _build(meta):
    lens = meta["lens"]
    calls = meta["calls"]
    n_slots = meta["n_slots"]
    n_win = meta["n_win"]
    gw = lens // P                       # windows per (chunk, group)
    win0 = meta["slot_off"] // P         # first window of (chunk, group)
    max_call_win = int(meta["max_call_win"])
    max_gw = int(gw.max())

    SIM = bool(os.environ.get("BIS_SIM"))
    nc = bacc.Bacc(
        "TRN2", target_bir_lowering=False, debug=False,
        num_devices=1 if SIM else NCORES,
        num_swdge_queues=4,
        dynamic_dma_scratch_size=49152,
    )

    # ---- I/O
    tab_in = [
        nc.dram_tensor(f"tab_in{c}", [PCHUNK, 2 * D], BF16, kind="ExternalInput")
        for c in range(NCHUNK)
    ]
    idx_in = nc.dram_tensor("idx_in", [16, n_slots // 16], mybir.dt.int16,
                            kind="ExternalInput")
    off_in = nc.dram_tensor("off_in", [P, n_win], BF16, kind="ExternalInput")
    xT_in = nc.dram_tensor("xT_in", [D, SHARD_N], BF16, kind="ExternalInput")
    scaleT_in = nc.dram_tensor("scaleT_in", [D, SHARD_N], BF16, kind="ExternalInput")
    wlb_in = [nc.dram_tensor(f"wlb{i}", [D + 1, D], BF16, kind="ExternalInput")
              for i in range(2)]
    wr_in = [nc.dram_tensor(f"wr{i}", [D, D], BF16, kind="ExternalInput")
             for i in range(2)]
    a_in = [nc.dram_tensor(f"a{i}", [D, P], BF16, kind="ExternalInput")
            for i in range(2)]
    out_ext = nc.dram_tensor("out", [D, SHARD_N], BF16, kind="ExternalOutput")

    with tile.TileContext(nc) as tc:
        with ExitStack() as ctx:
            dram = ctx.enter_context(tc.tile_pool(name="dram", bufs=1, space="DRAM"))
            const = ctx.enter_context(tc.tile_pool(name="const", bufs=1))
            gath = ctx.enter_context(tc.tile_pool(name="gath", bufs=1))
            spool = ctx.enter_context(tc.tile_pool(name="spool", bufs=2))
            epi = ctx.enter_context(tc.tile_pool(name="epi", bufs=4))
            ps_agg = ctx.enter_context(tc.tile_pool(name="ps_agg", bufs=3, space="PSUM"))
            ps_h = ctx.enter_context(tc.tile_pool(name="ps_h", bufs=2, space="PSUM"))
            ps_t = ctx.enter_context(tc.tile_pool(name="ps_t", bufs=1, space="PSUM"))
            ps_x = ctx.enter_context(tc.tile_pool(name="ps_x", bufs=1, space="PSUM"))

            # ---- persistent SBUF state
            off_sb = const.tile([P, n_win], BF16)
            nc.sync.dma_start(off_sb[:], off_in[:])
            # idx split into 4 call-aligned tiles
            # so the first gathers start before the whole index set loads
            ncol = n_slots // 16
            cum = 0
            bounds = [0]
            tgt = 1
            for (c_, g0_, ng_, s0_, L_) in calls:
                cum += L_ // 16
                if cum >= ncol * tgt // 4 and len(bounds) <= 3:
                    bounds.append(cum)
                    tgt += 1
            while len(bounds) < 5:
                bounds.append(ncol)
            bounds[4] = ncol
            idx_t = [
                const.tile([P, bounds[k + 1] - bounds[k]], mybir.dt.int16,
                           tag=f"idx{k}", name=f"idx{k}")
                for k in range(4)
            ]
            for k in range(4):
                for rep in range(8):
                    nc.sync.dma_start(
                        idx_t[k][16 * rep: 16 * (rep + 1), :],
                        idx_in[:, bounds[k]: bounds[k + 1]],
                    )

            def idx_view(col_lo, col_hi):
                for k in range(4):
                    if bounds[k] <= col_lo and col_hi <= bounds[k + 1]:
                        return idx_t[k][:, col_lo - bounds[k]: col_hi - bounds[k]]
                raise AssertionError("idx range straddles tiles")
            scaleT_sb = const.tile([D, SHARD_N], BF16)
            nc.sync.dma_start(scaleT_sb[:], scaleT_in[:])
            xT_sb = const.tile([D, SHARD_N], BF16)   # becomes agg0 after hxr0
            nc.sync.dma_start(xT_sb[:], xT_in[:])
            hxr_sb = [const.tile([D, SHARD_N], BF16, tag=f"hxr{i}", name=f"hxr{i}")
                      for i in range(2)]
            wlb_sb = [const.tile([D + 1, D], BF16, tag=f"wlb{i}", name=f"wlb_sb{i}")
                      for i in range(2)]
            wr_sb = [const.tile([D, D], BF16, tag=f"wr{i}", name=f"wr_sb{i}")
                     for i in range(2)]
            a_sb = [const.tile([D, P], BF16, tag=f"a{i}", name=f"a_sb{i}")
                    for i in range(2)]
            for i in range(2):
                nc.sync.dma_start(wlb_sb[i][:], wlb_in[i][:])
                nc.sync.dma_start(wr_sb[i][:], wr_in[i][:])
                nc.sync.dma_start(a_sb[i][:], a_in[i][:])
            ident = const.tile([P, P], BF16)
            make_identity(nc, ident[:])
            iota_i = const.tile([P, 256], mybir.dt.int32)
            nc.gpsimd.iota(iota_i[:], pattern=[[1, 256]], base=0, channel_multiplier=0)
            iota_sb = const.tile([P, 256], BF16)
            nc.vector.tensor_copy(iota_sb[:], iota_i[:])

            # sagg tiles with a constant ones row (bias); manual 4-buffer ring
            sagg_t = [const.tile([D + 1, P], BF16, tag=f"sg{i}", name=f"sagg{i}")
                      for i in range(4)]
            for t in sagg_t:
                nc.vector.memset(t[D: D + 1, :], 1.0)

            # DRAM: layer-1 table + AllGather input staging
            ag_in = [dram.tile([PAIRS_CC, 2 * D], BF16, tag=f"agi{c}", name=f"agi{c}")
                     for c in range(NCHUNK)]
            tab1 = [
                dram.tile([PCHUNK, 2 * D], BF16,
                          addr_space="Local" if SIM else "Shared",
                          tag=f"tab1_{c}", name=f"tab1_{c}")
                for c in range(NCHUNK)
            ]

            def gather_call(tab, call, q):
                c, g0, ng, s0, L = call
                ftb = gath.tile([P, max_call_win * 2 * D], BF16,
                                tag=f"ftb{q}", name=f"ftb{q}")
                nc.gpsimd.dma_gather(
                    out_ap=ftb[:].rearrange("p (w d) -> p w d", d=2 * D)[
                        :, 0: L // P, :
                    ],
                    in_ap=tab[:],
                    idxs_ap=idx_view(s0 // 16, (s0 + L) // 16),
                    num_idxs=L,
                    num_idxs_reg=L,
                    elem_size=2 * D,
                    single_packet=False,
                    queue_num=q,
                )
                return ftb

            def build_s(c, g):
                nw = int(gw[c, g])
                w0 = int(win0[c, g])
                s_t = spool.tile([P, max_gw * 256], BF16, tag="s", name="s_t")
                nc.vector.tensor_tensor(
                    out=s_t[:].rearrange("p (w q) -> p w q", q=256)[:, :nw, :],
                    in0=iota_sb[:].unsqueeze(1).broadcast_to([P, nw, 256]),
                    in1=off_sb[:, w0: w0 + nw].unsqueeze(2).broadcast_to(
                        [P, nw, 256]
                    ),
                    op=mybir.AluOpType.is_equal,
                )
                return s_t

            def scatter_mms(psum, ftb, s_t, c, g, g0call, start, stop):
                nw = int(gw[c, g])
                cbase = int(win0[c, g] - win0[c, g0call])
                fv = ftb[:].rearrange("p (w d) -> p w d", d=2 * D)
                sv = s_t[:].rearrange("p (w q) -> p w q", q=256)
                for w in range(nw):
                    nc.tensor.matmul(
                        psum[:], lhsT=fv[:, cbase + w, 0:D],
                        rhs=sv[:, w, 0:128],
                        start=(start and w == 0), stop=False,
                    )
                    nc.tensor.matmul(
                        psum[:], lhsT=fv[:, cbase + w, D:2 * D],
                        rhs=sv[:, w, 128:256],
                        start=False, stop=(stop and w == nw - 1),
                    )

            def build_hxr(layer, src_sb):
                CH = 512
                for k in range((SHARD_N + CH - 1) // CH):
                    lo = k * CH
                    hi = min(SHARD_N, lo + CH)
                    ps = ps_x.tile([D, CH], F32, space="PSUM")
                    nc.tensor.matmul(
                        ps[:, : hi - lo], lhsT=wr_sb[layer][:],
                        rhs=src_sb[:, lo:hi], start=True, stop=True,
                    )
                    nc.scalar.activation(
                        out=hxr_sb[layer][:, lo:hi], in_=ps[:, : hi - lo],
                        func=mybir.ActivationFunctionType.Copy,
                    )

            agg0_sb = xT_sb  # reused once xT is consumed by build_hxr(0, .)

            build_hxr(0, xT_sb[:])

            qn = 0
            for layer in range(2):
                for c in range(NCHUNK):
                    tab = tab_in[c] if layer == 0 else tab1[c]
                    for call in [cl for cl in calls if cl[0] == c]:
                        qn = (qn + 1) % 4
                        ftb = gather_call(tab, call, qn)
                        for g in range(call[1], call[1] + call[2]):
                            s_t = build_s(c, g)
                            if c == 0:
                                # partial aggregation pass
                                psum = ps_agg.tile([D, P], F32, space="PSUM")
                                scatter_mms(psum, ftb, s_t, c, g, call[1], True, True)
                                nc.scalar.activation(
                                    out=agg0_sb[:, g * P: (g + 1) * P], in_=psum[:],
                                    func=mybir.ActivationFunctionType.Copy,
                                )
                                continue
                            # chunk-1 pass: finish aggregation + epilogue
                            psum = ps_agg.tile([D, P], F32, space="PSUM")
                            scatter_mms(psum, ftb, s_t, c, g, call[1], True, False)
                            nc.tensor.matmul(
                                psum[:], lhsT=ident[0:D, 0:D],
                                rhs=agg0_sb[:, g * P: (g + 1) * P],
                                start=False, stop=True,
                            )
                            sagg = sagg_t[g % 4]
                            nc.vector.tensor_tensor(
                                out=sagg[0:D, :], in0=psum[:],
                                in1=scaleT_sb[:, g * P: (g + 1) * P],
                                op=mybir.AluOpType.mult,
                            )
                            hps = ps_h.tile([D, P], F32, space="PSUM")
                            nc.tensor.matmul(
                                hps[:], lhsT=wlb_sb[layer][:], rhs=sagg[:],
                                start=True, stop=False,
                            )
                            nc.tensor.matmul(
                                hps[:], lhsT=ident[0:D, 0:D],
                                rhs=hxr_sb[layer][:, g * P: (g + 1) * P],
                                start=False, stop=True,
                            )
                            # PReLU(v) = max(v, a*v)   (a in (0,1))
                            v_t = epi.tile([D, P], BF16, tag="v", name="v_t")
                            nc.scalar.activation(
                                out=v_t[:], in_=hps[:],
                                func=mybir.ActivationFunctionType.Copy,
                            )
                            t_t = epi.tile([D, P], BF16, tag="t", name="t_t")
                            nc.vector.tensor_tensor(
                                out=t_t[:], in0=v_t[:], in1=a_sb[layer][:],
                                op=mybir.AluOpType.mult,
                            )
                            if layer == 0:
                                h_t = epi.tile([D, P], BF16, tag="h", name="h_t")
                                nc.vector.tensor_max(h_t[:], v_t[:], t_t[:])
                                # layer-1 lin_r term for this group
                                pxx = ps_x.tile([D, P], F32, space="PSUM", tag="px")
                                nc.tensor.matmul(
                                    pxx[:], lhsT=wr_sb[1][:], rhs=h_t[:],
                                    start=True, stop=True,
                                )
                                nc.scalar.activation(
                                    out=hxr_sb[1][:, g * P: (g + 1) * P], in_=pxx[:],
                                    func=mybir.ActivationFunctionType.Copy,
                                )
                                # node-major copy for the AllGather table
                                tps = ps_t.tile([P, D], BF16, space="PSUM")
                                nc.tensor.transpose(
                                    out=tps[:], in_=h_t[:], identity=ident[0:D, 0:D]
                                )
                                ht_nm = epi.tile([P, D], BF16, tag="nm", name="ht_nm")
                                nc.scalar.activation(
                                    out=ht_nm[:], in_=tps[:],
                                    func=mybir.ActivationFunctionType.Copy,
                                )
                                cc = g // GC
                                r0 = (g % GC) * D
                                nc.sync.dma_start(
                                    ag_in[cc][r0: r0 + D, 0:D], ht_nm[0:D, :]
                                )
                                nc.sync.dma_start(
                                    ag_in[cc][r0: r0 + D, D:2 * D], ht_nm[D:P, :]
                                )
                                if g == GC - 1 or g == NGROUPS - 1:
                                    cc = 0 if g == GC - 1 else 1
                                    if SIM:
                                        for rep in range(NCORES):
                                            nc.sync.dma_start(
                                                tab1[cc][
                                                    rep * PAIRS_CC:
                                                    (rep + 1) * PAIRS_CC, :
                                                ],
                                                ag_in[cc][:],
                                            )
                                    else:
                                        nc.gpsimd.collective_compute(
                                            "AllGather",
                                            mybir.AluOpType.bypass,
                                            replica_groups=[list(range(NCORES))],
                                            ins=[ag_in[cc].opt()],
                                            outs=[tab1[cc].opt()],
                                        )
                            else:
                                o_t = epi.tile([D, P], BF16, tag="o", name="o_t")
                                nc.vector.tensor_max(o_t[:], v_t[:], t_t[:])
                                nc.sync.dma_start(
                                    out_ext[:, g * P: (g + 1) * P], o_t[:]
                                )

    nc.compile()
    return nc


# ------------------------------------------------------------------ runner

_CACHE = {}


def _get_program(edge_index):
    key = hash(
        (edge_index.shape, edge_index.dtype.str, edge_index[:, ::997].tobytes())
    )
    if key not in _CACHE:
        meta, percore = _prep(edge_index)
        nc = _build(meta)
        _CACHE[key] = (nc, meta, percore)
    return _CACHE[key]


def kernel(x, edge_index, w_l0, b_l0, w_r0, a0, w_l1, b_l1, w_r1, a1):
    import ml_dtypes

    x = np.asarray(x, dtype=np.float32)
    edge_index = np.asarray(edge_index)
    nc, meta, pc = _get_program(edge_index)

    node_core = pc["node_core"]
    node_local = pc["node_local"]
    g_of = pc["g_of"]
    p_of = pc["p_of"]

    xb = x.astype(ml_dtypes.bfloat16)
    # replicated pair-row tables per chunk
    tabs = [np.zeros((PCHUNK, 2 * D), ml_dtypes.bfloat16) for _ in range(NCHUNK)]
    chunk_n = g_of // GC
    rown = node_core * PAIRS_CC + (g_of % GC) * D + (p_of % D)
    parn = p_of // D
    for c in range(NCHUNK):
        m = chunk_n == c
        tabs[c][
            rown[m][:, None], (parn[m] * D)[:, None] + np.arange(D)[None, :]
        ] = xb[m]

    # per-core channel-major features
    xT = np.zeros((NCORES, D, SHARD_N), ml_dtypes.bfloat16)
    xT[node_core, :, node_local] = xb

    wlbs = []
    for wl, bl in ((w_l0, b_l0), (w_l1, b_l1)):
        wlbs.append(
            np.concatenate(
                [np.asarray(wl, np.float32), np.asarray(bl, np.float32)[None, :]], 0
            ).astype(ml_dtypes.bfloat16)
        )
    wrs = [np.asarray(w, np.float32).astype(ml_dtypes.bfloat16) for w in (w_r0, w_r1)]
    acols = [
        np.repeat(np.asarray(a, np.float32).reshape(D, 1), P, axis=1).astype(
            ml_dtypes.bfloat16
        )
        for a in (a0, a1)
    ]

    in_maps = []
    for c in range(NCORES):
        in_maps.append(
            {
                "tab_in0": tabs[0],
                "tab_in1": tabs[1],
                "idx_in": pc["idx_wrap"][c],
                "off_in": pc["offw"][c].astype(ml_dtypes.bfloat16),
                "xT_in": xT[c],
                "scaleT_in": pc["scaleT"][c],
                "wlb0": wlbs[0],
                "wlb1": wlbs[1],
                "wr0": wrs[0],
                "wr1": wrs[1],
                "a0": acols[0],
                "a1": acols[1],
            }
        )

    global _last_in_maps
    _last_in_maps = in_maps
    res = bass_utils.run_bass_kernel_spmd(nc, in_maps, core_ids=list(range(NCORES)))

    out = np.empty((N_NODES, D), np.float32)
    for c in range(NCORES):
        shard_out = np.asarray(res.results[c]["out"], dtype=np.float32)
        m = node_core == c
        out[m] = shard_out[:, node_local[m]].T
    return out
